# revision 7
# baseline (speedup 1.0000x reference)
"""AttnDecoderRNN (GRU x2 + Luong attention + vocab projection + log_softmax)
on 8 Trainium2 NeuronCores via Bass/Tile, SPMD with collectives.

Sharding (uniform across cores; core c owns hidden cols cH = [c*256,(c+1)*256)):
  - GRU gate matrices row-sharded (3 gates x 256 rows/core); h0 AllGathered.
  - Attention: v = h1 @ wa contraction-sharded -> ReduceScatter; energies
    contraction-sharded -> AllReduce; context computed per-core on enc[:, cH].
  - Vocab projection contraction-sharded over x = [h1 | context] (512 cols/core)
    -> AllReduce of full logits; log_softmax computed redundantly per core.
Energies use score associativity: rnn @ (enc @ Wa.T + ba).T == (rnn @ Wa) @ enc.T
+ const; the const shifts all energies equally so softmax is unchanged.

All GEMVs run on the tensor engine with host-pre-transposed weight blocks
(k-major), streaming weights from HBM at full DMA width.
"""
import numpy as np

import concourse.bass as bass
import concourse.mybir as mybir
import concourse.tile as tile
from concourse.bass_utils import run_bass_kernel_spmd

NC = 8
H = 2048
V = 50257
S = 2048
HQ = H // NC            # 256
HT = H // 128           # 16
VT = (V + 127) // 128   # 393
VPAD = VT * 128         # 50304
ST = S // 128           # 16
KX = 2 * HQ             # 512
NEG = -30000.0

F32 = mybir.dt.float32
ADD = mybir.AluOpType.add
SUB = mybir.AluOpType.subtract
MAX = mybir.AluOpType.max
EXP = mybir.ActivationFunctionType.Exp
LN = mybir.ActivationFunctionType.Ln
SIG = mybir.ActivationFunctionType.Sigmoid
TANH = mybir.ActivationFunctionType.Tanh


def _split_multiwaits(nc, limit=1):
    """walrus in this toolchain accepts at most one sem wait per instruction;
    move extra waits onto standalone EventSemaphore instructions."""
    ctr = 0
    for f in nc.m.functions:
        for bb in f.blocks:
            out = []
            changed = False
            for inst in bb.instructions:
                si = inst.sync_info
                if si is not None:
                    waits = list(si.on_wait)
                    if len(waits) > limit:
                        keep = waits[len(waits) - limit:]
                        for w in waits[:len(waits) - limit]:
                            ev = mybir.InstEventSemaphore(name=f"mwsplit-{ctr}")
                            ctr += 1
                            ev.engine = inst.engine
                            ev.sync_info = mybir.SyncInfo(on_wait=[w], on_update=[])
                            out.append(ev)
                        inst.sync_info = mybir.SyncInfo(
                            on_wait=keep, on_update=list(si.on_update))
                        changed = True
                out.append(inst)
            if changed:
                bb.instructions = out
    return ctr


def _build():
    nc = bass.Bass(num_devices=NC)

    # ---- I/O ----
    xe_d = nc.dram_tensor("xe", [128, HT], F32, kind="ExternalInput")      # col layout
    hp0c_d = nc.dram_tensor("hp0c", [128, HT], F32, kind="ExternalInput")
    hp1c_d = nc.dram_tensor("hp1c", [128, HT], F32, kind="ExternalInput")
    hp0s_d = nc.dram_tensor("hp0s", [HQ], F32, kind="ExternalInput")
    hp1s_d = nc.dram_tensor("hp1s", [HQ], F32, kind="ExternalInput")
    gw_d = {}
    gb_d = {}
    for l in (0, 1):
        for kind in ("ih", "hh"):
            # [HT][128, 6*128]: k-major transposed gate-weight blocks
            gw_d[l, kind] = nc.dram_tensor(f"w{kind}{l}", [HT, 128, 768], F32,
                                           kind="ExternalInput")
            gb_d[l, kind] = nc.dram_tensor(f"b{kind}{l}", [768], F32,
                                           kind="ExternalInput")
    was_d = nc.dram_tensor("was", [HQ, H], F32, kind="ExternalInput")     # rows of wa
    encs_d = nc.dram_tensor("encs", [S, HQ], F32, kind="ExternalInput")   # enc[:, cH]
    encts_d = nc.dram_tensor("encts", [HQ, S], F32, kind="ExternalInput")  # enc.T rows
    wouts_d = nc.dram_tensor("wouts", [VT, 128, KX], F32, kind="ExternalInput")
    bout_d = nc.dram_tensor("bout", [128, VT], F32, kind="ExternalInput")

    ls_o = nc.dram_tensor("ls", [128, VT], F32, kind="ExternalOutput")
    attn_o = nc.dram_tensor("attn", [S], F32, kind="ExternalOutput")
    h0s_o = nc.dram_tensor("h0s", [HQ], F32, kind="ExternalOutput")
    h1s_o = nc.dram_tensor("h1s", [HQ], F32, kind="ExternalOutput")

    groups = [list(range(NC))]

    with tile.TileContext(nc) as tc:
        with (
            tc.tile_pool(name="dram", bufs=1, space="DRAM") as dram,
            tc.tile_pool(name="gruw", bufs=3) as gruw,        # [128, 768] 3KB/p
            tc.tile_pool(name="big", bufs=2) as bigp,         # [128, 2048] 8KB/p
            tc.tile_pool(name="encp", bufs=4) as encp,        # [128, 256] 1KB/p
            tc.tile_pool(name="small", bufs=1) as small,
            tc.tile_pool(name="wp", bufs=44) as wp,           # [128, 512] 2KB/p
            tc.tile_pool(name="psg", bufs=1, space="PSUM") as psg,
            tc.tile_pool(name="psv", bufs=2, space="PSUM") as psv,
            tc.tile_pool(name="psx", bufs=2, space="PSUM") as psx,
            tc.tile_pool(name="psl", bufs=2, space="PSUM") as psl,
        ):
            # dram bounce buffers
            h0s_b = dram.tile([HQ], F32, tag="h0s_b")
            h0f_b = dram.tile([H], F32, tag="h0f_b")
            vpart_b = dram.tile([H], F32, tag="vpart_b")
            vs_b = dram.tile([HQ], F32, tag="vs_b")
            epart_b = dram.tile([S], F32, tag="epart_b")
            efull_b = dram.tile([S], F32, tag="efull_b")
            h1s_b = dram.tile([HQ], F32, tag="h1s_b")
            attn_b = dram.tile([S], F32, tag="attn_b")
            ctx_b = dram.tile([HQ], F32, tag="ctx_b")
            lpart_b = dram.tile([128, VT], F32, tag="lpart_b")
            lfull_b = dram.tile([128, VT], F32, tag="lfull_b")

            ones_row = small.tile([1, 128], F32, tag="ones_row")
            nc.vector.memset(ones_row[:], 1.0)
            ones_col = small.tile([128, 1], F32, tag="ones_col")
            nc.vector.memset(ones_col[:], 1.0)

            def gru_gemv(wdram, bias_sb, xcol_sb, out_sb):
                """out_sb[1,768] = bias + x @ W_shard.T  (contraction over H).

                lhsT = x column chunk (stationary), rhs = k-major weight block.
                The [1,768] psum spans two banks: cols 0:512 and 512:768 are
                separate accumulation groups in separate banks, so start=True
                bank-zeroing is safe."""
                ps = psg.tile([1, 768], F32, tag="psg")
                for kc in range(HT):
                    wt = gruw.tile([128, 768], F32, tag="gruw")
                    nc.sync.dma_start(out=wt[:], in_=wdram[kc])
                    nc.tensor.matmul(
                        out=ps[0:1, 0:512], lhsT=xcol_sb[:, kc:kc + 1],
                        rhs=wt[:, 0:512],
                        start=(kc == 0), stop=(kc == HT - 1),
                        skip_group_check=True)
                    nc.tensor.matmul(
                        out=ps[0:1, 512:768], lhsT=xcol_sb[:, kc:kc + 1],
                        rhs=wt[:, 512:768],
                        start=(kc == 0), stop=(kc == HT - 1),
                        skip_group_check=True)
                nc.vector.tensor_add(out_sb[:], ps[:], bias_sb[:])

            def gates(gi, gh, hps, hout):
                """rows [1, 768] -> hout [1, 256]"""
                r = small.tile([1, HQ], F32, tag="g_r")
                z = small.tile([1, HQ], F32, tag="g_z")
                n_ = small.tile([1, HQ], F32, tag="g_n")
                t1 = small.tile([1, HQ], F32, tag="g_t1")
                nc.vector.tensor_add(t1[:], gi[0:1, 0:HQ], gh[0:1, 0:HQ])
                nc.scalar.activation(r[:], t1[:], SIG)
                nc.vector.tensor_add(t1[:], gi[0:1, HQ:2 * HQ], gh[0:1, HQ:2 * HQ])
                nc.scalar.activation(z[:], t1[:], SIG)
                nc.vector.tensor_mul(t1[:], r[:], gh[0:1, 2 * HQ:768])
                nc.vector.tensor_add(t1[:], t1[:], gi[0:1, 2 * HQ:768])
                nc.scalar.activation(n_[:], t1[:], TANH)
                # h' = n + z*(hps - n)
                nc.vector.tensor_sub(t1[:], hps[:], n_[:])
                nc.vector.tensor_mul(t1[:], z[:], t1[:])
                nc.vector.tensor_add(hout[:], n_[:], t1[:])

            # ---- small loads ----
            bias_sb = {}
            for l in (0, 1):
                for kind in ("ih", "hh"):
                    bt = small.tile([1, 768], F32, tag=f"b{kind}{l}")
                    nc.sync.dma_start(out=bt[:], in_=gb_d[l, kind][:].unsqueeze(0))
                    bias_sb[l, kind] = bt
            hp0s_sb = small.tile([1, HQ], F32, tag="hp0s")
            nc.sync.dma_start(out=hp0s_sb[:], in_=hp0s_d[:].unsqueeze(0))
            hp1s_sb = small.tile([1, HQ], F32, tag="hp1s")
            nc.sync.dma_start(out=hp1s_sb[:], in_=hp1s_d[:].unsqueeze(0))
            xe_sb = small.tile([128, HT], F32, tag="xe_sb")
            nc.sync.dma_start(out=xe_sb[:], in_=xe_d[:])
            hp0c_sb = small.tile([128, HT], F32, tag="hp0c_sb")
            nc.sync.dma_start(out=hp0c_sb[:], in_=hp0c_d[:])
            hp1c_sb = small.tile([128, HT], F32, tag="hp1c_sb")
            nc.sync.dma_start(out=hp1c_sb[:], in_=hp1c_d[:])

            # ---- GRU layer 0 ----
            gi0 = small.tile([1, 768], F32, tag="gi0")
            gh0 = small.tile([1, 768], F32, tag="gh0")
            gru_gemv(gw_d[0, "ih"], bias_sb[0, "ih"], xe_sb, gi0)
            gru_gemv(gw_d[0, "hh"], bias_sb[0, "hh"], hp0c_sb, gh0)
            h0_sb = small.tile([1, HQ], F32, tag="h0_sb")
            gates(gi0, gh0, hp0s_sb, h0_sb)
            nc.sync.dma_start(out=h0s_b[:].unsqueeze(0), in_=h0_sb[:])
            nc.sync.dma_start(out=h0s_o[:].unsqueeze(0), in_=h0_sb[:])
            nc.gpsimd.collective_compute(
                "AllGather", mybir.AluOpType.bypass, replica_groups=groups,
                ins=[h0s_b[:].opt()], outs=[h0f_b[:].opt()])

            # ---- GRU layer 1 (x = h0_full, h = hp1) ----
            h0c_sb = small.tile([128, HT], F32, tag="h0c_sb")
            nc.sync.dma_start(out=h0c_sb[:], in_=h0f_b[:].rearrange("(t p) -> p t", p=128))
            gi1 = small.tile([1, 768], F32, tag="gi1")
            gh1 = small.tile([1, 768], F32, tag="gh1")
            gru_gemv(gw_d[1, "ih"], bias_sb[1, "ih"], h0c_sb, gi1)
            gru_gemv(gw_d[1, "hh"], bias_sb[1, "hh"], hp1c_sb, gh1)
            h1_sb = small.tile([1, HQ], F32, tag="h1_sb")
            gates(gi1, gh1, hp1s_sb, h1_sb)
            nc.sync.dma_start(out=h1s_b[:].unsqueeze(0), in_=h1_sb[:])
            nc.sync.dma_start(out=h1s_o[:].unsqueeze(0), in_=h1_sb[:])

            # ---- v_partial = h1_s @ wa[cH, :]  -> ReduceScatter ----
            wa_t = []
            for m in range(2):
                wt = bigp.tile([128, H], F32, tag="bigw")
                nc.sync.dma_start(out=wt[:], in_=was_d[m * 128:(m + 1) * 128, :])
                wa_t.append(wt)
            h1_col = small.tile([128, 2], F32, tag="h1_col")
            nc.sync.dma_start(out=h1_col[:],
                              in_=h1s_b[:].rearrange("(m p) -> p m", p=128))
            v_row = small.tile([1, H], F32, tag="v_row")
            for n in range(4):
                pv = psv.tile([1, 512], F32, tag="pv")
                for m in range(2):
                    nc.tensor.matmul(
                        out=pv[:], lhsT=h1_col[:, m:m + 1],
                        rhs=wa_t[m][:, n * 512:(n + 1) * 512],
                        start=(m == 0), stop=(m == 1))
                nc.vector.tensor_copy(v_row[0:1, n * 512:(n + 1) * 512], pv[0:1, :])
            nc.sync.dma_start(out=vpart_b[:].unsqueeze(0), in_=v_row[:])
            nc.gpsimd.collective_compute(
                "ReduceScatter", ADD, replica_groups=groups,
                ins=[vpart_b[:].opt()], outs=[vs_b[:].opt()])

            # ---- energies_partial = v[cH] @ enc.T[cH, :] -> AllReduce ----
            v_col = small.tile([128, 2], F32, tag="v_col")
            nc.sync.dma_start(out=v_col[:], in_=vs_b[:].rearrange("(m p) -> p m", p=128))
            enct_t = []
            for m in range(2):
                wt = bigp.tile([128, S], F32, tag="bigw")
                nc.sync.dma_start(out=wt[:], in_=encts_d[m * 128:(m + 1) * 128, :])
                enct_t.append(wt)
            e_row = small.tile([1, S], F32, tag="e_row")
            for n in range(4):
                pv = psv.tile([1, 512], F32, tag="pv")
                for m in range(2):
                    nc.tensor.matmul(
                        out=pv[:], lhsT=v_col[:, m:m + 1],
                        rhs=enct_t[m][:, n * 512:(n + 1) * 512],
                        start=(m == 0), stop=(m == 1))
                nc.vector.tensor_copy(e_row[0:1, n * 512:(n + 1) * 512], pv[0:1, :])
            nc.sync.dma_start(out=epart_b[:].unsqueeze(0), in_=e_row[:])
            nc.gpsimd.collective_compute(
                "AllReduce", ADD, replica_groups=groups,
                ins=[epart_b[:].opt()], outs=[efull_b[:].opt()])

            # ---- softmax over S on a single row ----
            ef_row = small.tile([1, S], F32, tag="ef_row")
            nc.sync.dma_start(out=ef_row[:], in_=efull_b[:].unsqueeze(0))
            rmax = small.tile([1, 1], F32, tag="rmax")
            nc.vector.tensor_reduce(out=rmax[:], in_=ef_row[:],
                                    axis=mybir.AxisListType.X, op=MAX)
            nmax = small.tile([1, 1], F32, tag="nmax")
            nc.vector.tensor_scalar_mul(nmax[:], rmax[:], -1.0)
            exp_row = small.tile([1, S], F32, tag="exp_row")
            esum = small.tile([1, 1], F32, tag="esum")
            nc.scalar.activation(exp_row[:], ef_row[:], EXP,
                                 bias=nmax[0:1, 0:1], scale=1.0, accum_out=esum[:])
            einv = small.tile([1, 1], F32, tag="einv")
            nc.vector.reciprocal(einv[:], esum[:])
            attn_row = small.tile([1, S], F32, tag="attn_row")
            nc.vector.tensor_scalar_mul(attn_row[:], exp_row[:], einv[0:1, 0:1])
            nc.sync.dma_start(out=attn_o[:].unsqueeze(0), in_=attn_row[:])
            nc.sync.dma_start(out=attn_b[:].unsqueeze(0), in_=attn_row[:])

            # ---- context = attn @ enc[:, cH] ----
            attn_col = small.tile([128, ST], F32, tag="attn_col")
            nc.sync.dma_start(out=attn_col[:],
                              in_=attn_b[:].rearrange("(t p) -> p t", p=128))
            pctx_t = psv.tile([1, 512], F32, tag="pv")
            pctx = pctx_t[0:1, 0:HQ]
            for t in range(ST):
                et = encp.tile([128, HQ], F32, tag="enc")
                nc.sync.dma_start(out=et[:], in_=encs_d[t * 128:(t + 1) * 128, :])
                nc.tensor.matmul(out=pctx, lhsT=attn_col[:, t:t + 1], rhs=et[:],
                                 start=(t == 0), stop=(t == ST - 1))
            ctx_row = small.tile([1, HQ], F32, tag="ctx_row")
            nc.vector.tensor_copy(ctx_row[:], pctx)
            nc.sync.dma_start(out=ctx_b[:].unsqueeze(0), in_=ctx_row[:])
            ctx_col = small.tile([128, 2], F32, tag="ctx_col")
            nc.sync.dma_start(out=ctx_col[:],
                              in_=ctx_b[:].rearrange("(m p) -> p m", p=128))

            # ---- logits_partial: w_out[:, local 512 cols] @ x_local ----
            logits_sb = small.tile([128, VT], F32, tag="logits_sb")
            xc = [h1_col[:, 0:1], h1_col[:, 1:2], ctx_col[:, 0:1], ctx_col[:, 1:2]]
            for t in range(VT):
                wt = wp.tile([128, KX], F32, tag="w")
                nc.sync.dma_start(out=wt[:], in_=wouts_d[t])
                lp = psl.tile([128, 1], F32, tag="lp")
                for kc in range(4):
                    nc.tensor.matmul(
                        out=lp[:], lhsT=wt[:, kc * 128:(kc + 1) * 128], rhs=xc[kc],
                        start=(kc == 0), stop=(kc == 3))
                nc.vector.tensor_copy(logits_sb[:, t:t + 1], lp[:])
            nc.sync.dma_start(out=lpart_b[:], in_=logits_sb[:])
            nc.gpsimd.collective_compute(
                "AllReduce", ADD, replica_groups=groups,
                ins=[lpart_b[:].opt()], outs=[lfull_b[:].opt()])

            # ---- log_softmax over V ----
            lf_sb = small.tile([128, VT], F32, tag="lf_sb")
            nc.sync.dma_start(out=lf_sb[:], in_=lfull_b[:])
            bout_sb = small.tile([128, VT], F32, tag="bout_sb")
            nc.sync.dma_start(out=bout_sb[:], in_=bout_d[:])
            nc.vector.tensor_add(lf_sb[:], lf_sb[:], bout_sb[:])
            lm = small.tile([128, 1], F32, tag="lm")
            nc.vector.tensor_reduce(out=lm[:], in_=lf_sb[:],
                                    axis=mybir.AxisListType.X, op=MAX)
            gmax = small.tile([1, 1], F32, tag="gmax")
            nc.gpsimd.tensor_reduce(out=gmax[:], in_=lm[:],
                                    axis=mybir.AxisListType.C, op=MAX)
            pb = psx.tile([128, 1], F32, tag="px")
            nc.tensor.matmul(out=pb[:], lhsT=ones_row[:], rhs=gmax[:],
                             start=True, stop=True)
            nmax_l = small.tile([128, 1], F32, tag="nmax_l")
            nc.vector.tensor_scalar_mul(nmax_l[:], pb[:], -1.0)
            lexp = small.tile([128, VT], F32, tag="lexp")
            lsum = small.tile([128, 1], F32, tag="lsum")
            nc.scalar.activation(lexp[:], lf_sb[:], EXP,
                                 bias=nmax_l[:, 0:1], scale=1.0, accum_out=lsum[:])
            pt = psx.tile([128, 1], F32, tag="px")
            nc.tensor.matmul(out=pt[0:1, 0:1], lhsT=lsum[:], rhs=ones_col[:],
                             start=True, stop=True)
            lln = small.tile([1, 1], F32, tag="lln")
            nc.scalar.activation(lln[:], pt[0:1, 0:1], LN)
            ofs1 = small.tile([1, 1], F32, tag="ofs1")
            nc.vector.tensor_add(ofs1[:], gmax[:], lln[:])
            pb2 = psx.tile([128, 1], F32, tag="px")
            nc.tensor.matmul(out=pb2[:], lhsT=ones_row[:], rhs=ofs1[:],
                             start=True, stop=True)
            ofs_sb = small.tile([128, 1], F32, tag="ofs_sb")
            nc.vector.tensor_copy(ofs_sb[:], pb2[:])
            nc.vector.tensor_scalar(out=lf_sb[:], in0=lf_sb[:],
                                    scalar1=ofs_sb[:, 0:1], scalar2=None, op0=SUB)
            nc.sync.dma_start(out=ls_o[:], in_=lf_sb[:])

    _split_multiwaits(nc)
    return nc


_NC_CACHE = None


def _get_nc():
    global _NC_CACHE
    if _NC_CACHE is None:
        _NC_CACHE = _build()
    return _NC_CACHE


def _host_prep(inputs):
    emb = np.asarray(inputs["emb"], dtype=np.float32)
    w = int(np.asarray(inputs["word_input"]).reshape(-1)[0])
    xe = np.ascontiguousarray(emb[w])
    lh = np.asarray(inputs["last_hidden"], np.float32)
    hp0 = np.ascontiguousarray(lh[0, 0])
    hp1 = np.ascontiguousarray(lh[1, 0])
    enc = np.ascontiguousarray(np.asarray(inputs["encoder_outputs"], np.float32)[:, 0, :])
    encT = np.ascontiguousarray(enc.T)
    wa = np.asarray(inputs["wa"], np.float32)
    w_out = np.asarray(inputs["w_out"], np.float32)
    b_out = np.asarray(inputs["b_out"], np.float32)
    bpad = np.full(VPAD, NEG, np.float32)
    bpad[:V] = b_out
    bout_t = np.ascontiguousarray(bpad.reshape(VT, 128).T)

    def col(hvec):
        return np.ascontiguousarray(hvec.reshape(HT, 128).T)   # [128, HT]

    def shard_slice(h, c):
        return np.ascontiguousarray(h[c * HQ:(c + 1) * HQ])

    def gate_wt(m, c):
        # rows [cH, H+cH, 2H+cH] -> ws [768, H]; block[kc, p, n] = ws[n, kc*128+p]
        ws = np.concatenate(
            [m[g * H + c * HQ:g * H + (c + 1) * HQ] for g in range(3)], axis=0)
        t = ws.reshape(768, HT, 128).transpose(1, 2, 0)
        return np.ascontiguousarray(t)

    def gate_bias(b, c):
        return np.ascontiguousarray(np.concatenate(
            [b[g * H + c * HQ:g * H + (c + 1) * HQ] for g in range(3)]))

    gmats = {(l, k): np.asarray(inputs[f"w_{k}{l}"], np.float32)
             for l in (0, 1) for k in ("ih", "hh")}
    gbias = {(l, k): np.asarray(inputs[f"b_{k}{l}"], np.float32)
             for l in (0, 1) for k in ("ih", "hh")}

    xec, hp0c, hp1c = col(xe), col(hp0), col(hp1)
    maps = []
    for c in range(NC):
        cs = slice(c * HQ, (c + 1) * HQ)
        # w_out local cols -> [VT][128(p=k%128), 4*128(kc,m)] k-major blocks
        wl = np.concatenate([w_out[:, cs], w_out[:, H + c * HQ:H + (c + 1) * HQ]],
                            axis=1)                      # [V, 512]
        wl = np.concatenate([wl, np.zeros((VPAD - V, KX), np.float32)], axis=0)
        # wl[t*128+m, kc*128+p] -> wh[t, p, kc, m]
        wh = np.ascontiguousarray(
            wl.reshape(VT, 128, 4, 128).transpose(0, 3, 2, 1)).reshape(VT, 128, KX)
        m = dict(
            xe=xec, hp0c=hp0c, hp1c=hp1c,
            hp0s=shard_slice(hp0, c), hp1s=shard_slice(hp1, c),
            was=np.ascontiguousarray(wa[cs, :]),
            encs=np.ascontiguousarray(enc[:, cs]),
            encts=np.ascontiguousarray(encT[cs, :]),
            wouts=wh, bout=bout_t,
        )
        for l in (0, 1):
            for k in ("ih", "hh"):
                m[f"w{k}{l}"] = gate_wt(gmats[l, k], c)
                m[f"b{k}{l}"] = gate_bias(gbias[l, k], c)
        maps.append(m)
    return maps


def _assemble(res):
    out = res[0]["ls"].T.reshape(VPAD)[:V].reshape(1, V).astype(np.float32)
    hidden = np.stack([
        np.concatenate([res[c]["h0s"] for c in range(NC)]).reshape(1, H),
        np.concatenate([res[c]["h1s"] for c in range(NC)]).reshape(1, H),
    ]).astype(np.float32)
    attn = res[0]["attn"].reshape(1, 1, S).astype(np.float32)
    return out, hidden, attn


def kernel(**inputs):
    nc = _get_nc()
    maps = _host_prep(inputs)
    res = run_bass_kernel_spmd(nc, maps, list(range(NC))).results
    return _assemble(res)
